# revision 22
# baseline (speedup 1.0000x reference)
"""Multi-head self-attention (B=4, N=2048, D=1024, H=16) on 8 trn2 NeuronCores.

Sharding: 8 shards = (batch, query-half).  Core c handles batch c//2 and query
rows [(c%2)*1024, (c%2)*1024+1024).  Each core receives its batch's z with the
rows rolled so that its query rows come first; rolling permutes the key/value
sequence order, which attention output is invariant to.  K/V are computed for
the full 2048-row sequence on both cores of a batch pair (duplicated compute,
no collectives needed).

Per-core kernel (Tile), restructured for PE/ACT overlap:
  - Everything SBUF-resident in bf16 (no DRAM spill of K^T/Q^T).
  - Per head-pair pipeline: projections for pair p are interleaved with
    attention for pair p-1 so the PE never idles while ACT drains the exp
    stream.
  - Scores via 64-row lhsT slices of the pair's K^T (no zero padding);
    per-head psum scores tile [128, 1024] -> one ACT exp instr.
  - PV in natural orientation: lhsT = exp-scores [128 keys, 128 q] slices,
    rhs = [V_h | 1] (65 cols) -> psum [128 q, 65] accumulated over key
    chunks; col 64 is the softmax denominator.  This streams 65 columns per
    accumulation step instead of 1024, halving PE attention work.
  - Normalize with per-partition reciprocal scalars on DVE, final projection
    from a PE re-transpose of the normalized attention output.
"""

import os
import sys

_TRN_REPO = "/opt/trn_rl_repo"
if os.path.isdir(_TRN_REPO) and _TRN_REPO not in sys.path:
    sys.path.insert(0, _TRN_REPO)

import numpy as np

import concourse.bass as bass  # noqa: E402
import concourse.mybir as mybir  # noqa: E402
from concourse import bacc  # noqa: E402
from concourse.bass_utils import run_bass_kernel_spmd  # noqa: E402
from concourse.masks import make_identity  # noqa: E402
from concourse.tile import TileContext  # noqa: E402

F32 = mybir.dt.float32
BF16 = mybir.dt.bfloat16
MULT = mybir.AluOpType.mult
ADD = mybir.AluOpType.add
EXP = mybir.ActivationFunctionType.Exp

N_CORES = 8
B, N, D = 4, 2048, 1024
H, HD = 16, 64
NQ = N // 2  # query rows per core
P = 128
DC = D // P  # 8 din/dout chunks of 128
NKC = N // P  # 16 key chunks of 128
NP = H // 2  # 8 head pairs
SCALE = 1.0 / 8.0  # 1/sqrt(HD)


def _build():
    nc = bacc.Bacc("TRN2", target_bir_lowering=False, debug=False,
                   num_devices=N_CORES)
    z_d = nc.declare_dram_parameter("z", [N, D], F32, isOutput=False)
    wq_d = nc.declare_dram_parameter("w_q", [D, D], F32, isOutput=False)
    wk_d = nc.declare_dram_parameter("w_k", [D, D], F32, isOutput=False)
    wv_d = nc.declare_dram_parameter("w_v", [D, D], F32, isOutput=False)
    wo_d = nc.declare_dram_parameter("w_o", [D, D], F32, isOutput=False)
    bo_d = nc.declare_dram_parameter("b_o", [D], F32, isOutput=False)
    out_d = nc.declare_dram_parameter("out", [NQ, D], F32, isOutput=True)
    # per-pair K^T/V' exchange scratch: [K h0 | K h1 | V'] own key-half
    KVW = 2 * NQ + NKC // 2 * 2 * (HD + 1)  # 3088 bf16 per partition
    kv_in = nc.dram_tensor("kv_in", [NP, P, KVW], BF16)
    kv_out = nc.dram_tensor("kv_out", [NP, 2, P, KVW], BF16)

    with TileContext(nc) as tc:
        with tc.tile_pool(name="const", bufs=1) as constp, \
             tc.tile_pool(name="pers", bufs=1) as persp:
            identf = constp.tile([P, P], F32, name="identf")
            make_identity(nc, identf)
            ident16 = constp.tile([P, P], BF16, name="ident16")
            make_identity(nc, ident16)

            bo_sb = constp.tile([1, D], F32, name="bo_sb")
            nc.sync.dma_start(bo_sb[:], bo_d[None, :])
            bias_bc = constp.tile([P, D], F32, name="bias_bc")
            nc.gpsimd.partition_broadcast(bias_bc[:], bo_sb[:])

            # attention output, natural ([q-part, qc, din]) and transposed
            attnN = persp.tile([P, NQ // P, D], BF16, name="attnN")
            attnT = persp.tile([P, DC, NQ], BF16, name="attnT")
            wo16 = persp.tile([P, DC, D], BF16, name="wo16")

            with tc.tile_pool(name="zts", bufs=1) as ztsp, \
                 tc.tile_pool(name="wpair", bufs=2) as wpairp, \
                 tc.tile_pool(name="wstg", bufs=2) as wstgp, \
                 tc.tile_pool(name="wostg", bufs=1) as wostgp, \
                 tc.tile_pool(name="zin", bufs=2) as zinp, \
                 tc.tile_pool(name="kqv", bufs=1) as kqvp, \
                 tc.tile_pool(name="es", bufs=6) as esp, \
                 tc.tile_pool(name="rec", bufs=4) as recp, \
                 tc.tile_pool(name="psproj", bufs=2, space="PSUM") as projps, \
                 tc.tile_pool(name="psscore", bufs=2, space="PSUM") as scoreps, \
                 tc.tile_pool(name="pspv", bufs=2, space="PSUM") as pvps:

                zT = ztsp.tile([P, DC, NQ], BF16, name="zT")

                # persistent, manually double-buffered K^T/Q^T/V' tiles.
                # K^T/Q^T are per head, zero-padded to 128 contraction rows
                # (head 0 at partitions 0:64, head 1 at 64:128); the pad and
                # the V' ones-column are memset ONCE here, off the critical
                # path, instead of per pair.
                kbuf = [[kqvp.tile([P, N], BF16, name="kb", tag=f"kb{i}{hh}")
                         for hh in range(2)] for i in range(2)]
                qbuf = [[kqvp.tile([P, NQ], BF16, name="qb", tag=f"qb{i}{hh}")
                         for hh in range(2)] for i in range(2)]
                vbuf = [kqvp.tile([P, NKC, 2, HD + 1], BF16, name="vb",
                                  tag=f"vb{i}") for i in range(2)]
                for i in range(2):
                    nc.gpsimd.memset(kbuf[i][0][HD:P, :], 0.0)
                    nc.gpsimd.memset(kbuf[i][1][0:HD, :], 0.0)
                    nc.gpsimd.memset(qbuf[i][0][HD:P, :], 0.0)
                    nc.gpsimd.memset(qbuf[i][1][0:HD, :], 0.0)
                    nc.gpsimd.memset(vbuf[i][:, :, :, HD], 1.0)

                # ---- z load + cast + bf16 transpose into zT ----
                def zt_unit(ch):
                    def f():
                        zin = zinp.tile([P, D], F32, name="zin", tag="zin")
                        nc.sync.dma_start(zin[:], z_d[ch * P:(ch + 1) * P, :])
                        zc = zinp.tile([P, D], BF16, name="zc", tag="zc")
                        nc.scalar.activation(
                            zc[:], zin[:], mybir.ActivationFunctionType.Copy)
                        for dg in range(2):
                            ps = projps.tile([P, 4, P], BF16, name="zps",
                                             tag="pp")
                            for d4 in range(4):
                                dc = dg * 4 + d4
                                nc.tensor.transpose(
                                    ps[:, d4, :],
                                    zc[:, dc * P:(dc + 1) * P],
                                    ident16[:])
                            nc.vector.tensor_copy(
                                zT[:, dg * 4:(dg + 1) * 4,
                                   ch * P:(ch + 1) * P],
                                ps[:])
                    return f

                # ---------- per-pair unit emitters ----------
                def make_proj_units(p, state):
                    """Projection chains for pair p (emitted lazily), fed by
                    per-pair weight column slices so pair 0 starts early."""
                    units = []
                    kTh, qTh = kbuf[p % 2], qbuf[p % 2]
                    vONp = vbuf[p % 2]
                    state[p] = (kTh, qTh, vONp)
                    w16 = {}

                    def w_load(nm, src_d):
                        def f():
                            stg = wstgp.tile([P, DC, P], F32, name="wstg",
                                             tag="wstg")
                            nc.sync.dma_start(
                                stg[:],
                                src_d[:, p * P:(p + 1) * P].rearrange(
                                    "(c p) o -> p c o", p=P))
                            w16[nm] = wpairp.tile([P, DC, P], BF16,
                                                  name="w16", tag=f"w{nm}")
                            nc.vector.tensor_copy(w16[nm][:], stg[:])
                        return f

                    def k_chain(s5):
                        def f():
                            ps = projps.tile([P, 512], F32, name="kps",
                                             tag="pp")
                            for dc in range(DC):
                                nc.tensor.matmul(
                                    ps[:],
                                    lhsT=w16["k"][:, dc, :],
                                    rhs=zT[:, dc, s5 * 512:(s5 + 1) * 512],
                                    start=(dc == 0), stop=(dc == DC - 1))
                            nc.vector.tensor_copy(
                                kTh[0][0:HD, s5 * 512:(s5 + 1) * 512],
                                ps[0:HD, :])
                            nc.vector.tensor_copy(
                                kTh[1][HD:P, s5 * 512:(s5 + 1) * 512],
                                ps[HD:P, :])
                        return f

                    def q_chain(s5):
                        def f():
                            ps = projps.tile([P, 512], F32, name="qps",
                                             tag="pp")
                            for dc in range(DC):
                                nc.tensor.matmul(
                                    ps[:],
                                    lhsT=w16["q"][:, dc, :],
                                    rhs=zT[:, dc, s5 * 512:(s5 + 1) * 512],
                                    start=(dc == 0), stop=(dc == DC - 1))
                            nc.vector.tensor_copy(
                                qTh[0][0:HD, s5 * 512:(s5 + 1) * 512],
                                ps[0:HD, :])
                            nc.vector.tensor_copy(
                                qTh[1][HD:P, s5 * 512:(s5 + 1) * 512],
                                ps[HD:P, :])
                        return f

                    def v_group(g):
                        def f():
                            vt = projps.tile([P, 4, P], F32, name="vps",
                                             tag="pp")
                            for kc4 in range(4):
                                kc = g * 4 + kc4
                                for dc in range(DC):
                                    nc.tensor.matmul(
                                        vt[:, kc4, :],
                                        lhsT=zT[:, dc, kc * P:(kc + 1) * P],
                                        rhs=w16["v"][:, dc, :],
                                        start=(dc == 0), stop=(dc == DC - 1))
                            nc.vector.tensor_copy(
                                vONp[:, g * 4:(g + 1) * 4, :, 0:HD],
                                vt.rearrange("p k (h d) -> p k h d", d=HD))
                        return f

                    def xfer_unit():
                        nc.sync.dma_start(kv_in[p, :, 0:NQ],
                                          kTh[0][:, 0:NQ])
                        nc.sync.dma_start(kv_in[p, :, NQ:2 * NQ],
                                          kTh[1][:, 0:NQ])
                        nc.sync.dma_start(
                            kv_in[p, :, 2 * NQ:].rearrange(
                                "p (k h d) -> p k h d", h=2, d=HD + 1),
                            vONp[:, 0:NKC // 2, :, :])

                    def cc_unit():
                        nc.gpsimd.collective_compute(
                            "AllGather", mybir.AluOpType.bypass,
                            replica_groups=[[0, 1], [2, 3], [4, 5], [6, 7]],
                            ins=[kv_in[p]], outs=[kv_out[p]])

                    def read_unit():
                        for m in range(2):
                            nc.sync.dma_start(
                                kTh[0][:, m * NQ:(m + 1) * NQ],
                                kv_out[p, m, :, 0:NQ])
                            nc.sync.dma_start(
                                kTh[1][:, m * NQ:(m + 1) * NQ],
                                kv_out[p, m, :, NQ:2 * NQ])
                            nc.sync.dma_start(
                                vONp[:, m * (NKC // 2):(m + 1) * (NKC // 2),
                                     :, :],
                                kv_out[p, m, :, 2 * NQ:].rearrange(
                                    "p (k h d) -> p k h d", h=2, d=HD + 1))

                    units.append(w_load("k", wk_d))
                    units.append(w_load("q", wq_d))
                    units.append(w_load("v", wv_d))
                    for s5 in range(2):
                        units.append(k_chain(s5))
                    for s5 in range(2):
                        units.append(q_chain(s5))
                    for g in range(2):
                        units.append(v_group(g))
                    units.append(xfer_unit)
                    units.append(cc_unit)
                    units.append(read_unit)
                    return units

                def make_attn_units(p, state):
                    """Attention stream for pair p: per head 16 x (S, exp,
                    PV one kc behind) + normalize, then attnT transposes for
                    this pair's din columns."""
                    kTh, qTh, vONp = state[p]
                    units = []

                    def pv_step(kc, hh, pv_tiles, es_tiles):
                        es = es_tiles.pop(kc)
                        for qc in range(8):
                            # one accumulation buffer per psum bank: only
                            # the bank's first chain may zero it (start);
                            # siblings accumulate onto the zeroed buffer.
                            nc.tensor.matmul(
                                pv_tiles[qc // 4][:, qc % 4, :],
                                lhsT=es[:, qc * P:(qc + 1) * P],
                                rhs=vONp[:, kc, hh, :],
                                start=(kc == 0 and qc % 4 == 0),
                                stop=(kc == NKC - 1),
                                skip_group_check=True)

                    for hh in range(2):
                        kTp, qTp = kTh[hh], qTh[hh]
                        pv_tiles = []
                        es_tiles = {}

                        def head_setup(pv_tiles=pv_tiles):
                            for _ in range(2):
                                pv_tiles.append(pvps.tile(
                                    [P, 4, HD + 1], F32, name="pv", tag="pv"))

                        def kc_unit(kc, hh=hh, kTp=kTp, qTp=qTp,
                                    pv_tiles=pv_tiles, es_tiles=es_tiles):
                            def f():
                                ps = scoreps.tile([P, NQ], F32, name="sps",
                                                  tag="sc")
                                for qc2 in range(2):
                                    nc.tensor.matmul(
                                        ps[:, qc2 * 512:(qc2 + 1) * 512],
                                        lhsT=kTp[:, kc * P:(kc + 1) * P],
                                        rhs=qTp[:,
                                                qc2 * 512:(qc2 + 1) * 512])
                                es = esp.tile([P, NQ], BF16, name="es",
                                              tag="es")
                                nc.scalar.activation(es[:], ps[:], EXP,
                                                     scale=SCALE)
                                es_tiles[kc] = es
                                if kc > 0:
                                    # PV runs one kc behind its exp so the
                                    # PE never waits on ACT just-in-time
                                    pv_step(kc - 1, hh, pv_tiles, es_tiles)
                            return f

                        def pv_tail(hh=hh, pv_tiles=pv_tiles,
                                    es_tiles=es_tiles):
                            def f():
                                pv_step(NKC - 1, hh, pv_tiles, es_tiles)
                            return f

                        def norm_unit(half, p=p, hh=hh, pv_tiles=pv_tiles):
                            def f():
                                pv = pv_tiles[half]
                                rec = recp.tile([P, 4, 1], F32, name="rec",
                                                tag="rec")
                                nc.vector.reciprocal(
                                    rec[:], pv[:, :, HD:HD + 1])
                                for qc4 in range(4):
                                    qc = half * 4 + qc4
                                    nc.vector.tensor_scalar(
                                        attnN[:, qc,
                                              (2 * p + hh) * HD:
                                              (2 * p + hh + 1) * HD],
                                        pv[:, qc4, 0:HD],
                                        rec[:, qc4, :], None, MULT)
                            return f

                        units.append(head_setup)
                        for kc in range(NKC):
                            units.append(kc_unit(kc))
                        units.append(pv_tail())
                        units.append(norm_unit(0))
                        units.append(norm_unit(1))

                    def at_unit(qg, p=p):
                        def f():
                            tp = projps.tile([P, 4, P], BF16, name="tp",
                                             tag="pp")
                            for q4 in range(4):
                                qc = qg * 4 + q4
                                nc.tensor.transpose(
                                    tp[:, q4, :],
                                    attnN[:, qc, p * P:(p + 1) * P],
                                    ident16[:])
                            nc.vector.tensor_copy(
                                attnT[:, p, qg * 512:(qg + 1) * 512]
                                .rearrange("p (q c) -> p q c", c=P),
                                tp[:])
                        return f

                    units.append(at_unit(0))
                    units.append(at_unit(1))
                    return units

                # ---------- pipelined emission ----------
                state = {}
                zt_units = [zt_unit(ch) for ch in range(8)]
                for p in range(NP + 1):
                    proj_units = make_proj_units(p, state) if p < NP else []
                    attn_units = make_attn_units(p - 1, state) if p > 0 else []
                    if not attn_units:
                        # pair 0 fill: z chunks interleaved with pair-0
                        # chains in dependency order (chain s5 needs z
                        # chunks 4*s5..4*s5+3)
                        wl, ch_u = proj_units[0:3], proj_units[3:]
                        K, Q, V = ch_u[0:2], ch_u[2:4], ch_u[4:6]
                        xf = proj_units[9:12]
                        order = ([wl[0]] + zt_units[0:4] + [K[0], wl[1]] +
                                 zt_units[4:8] + [K[1], Q[0], wl[2],
                                                  Q[1], V[0], V[1]] + xf)
                        for u in order:
                            u()
                        continue
                    # interleave: chains spread over the first ~55% of
                    # the attention stream, exchange units right after, so
                    # the AllGather completes well before attn(p) needs it
                    na, npj = len(attn_units), len(proj_units)
                    span = max(1, int(na * 0.55))
                    pi = 0
                    for i, u in enumerate(attn_units):
                        u()
                        want = npj if i >= span else ((i + 1) * npj) // span
                        while pi < want:
                            proj_units[pi]()
                            pi += 1
                    while pi < npj:
                        proj_units[pi]()
                        pi += 1
                    if p == 5:
                        # preload + cast w_o while attention still runs
                        for half in range(2):
                            stg = wostgp.tile([P, DC // 2, D], F32,
                                              name="wostg", tag="wostg")
                            nc.sync.dma_start(
                                stg[:],
                                wo_d[half * 512:(half + 1) * 512, :]
                                .rearrange("(c p) o -> p c o", p=P))
                            nc.vector.tensor_copy(
                                wo16[:, half * 4:(half + 1) * 4, :], stg[:])

            # ---------------- tail: final projection ----------
            with tc.tile_pool(name="ot", bufs=4) as outp, \
                 tc.tile_pool(name="psf", bufs=2, space="PSUM") as fpp:
                for qc in range(NQ // P):
                    for oc2 in range(2):
                        po = fpp.tile([P, 512], F32, name="po", tag="po")
                        for dc in range(DC):
                            nc.tensor.matmul(
                                po[:],
                                lhsT=attnT[:, dc, qc * P:(qc + 1) * P],
                                rhs=wo16[:, dc, oc2 * 512:(oc2 + 1) * 512],
                                start=(dc == 0), stop=(dc == DC - 1))
                        ot = outp.tile([P, 512], F32, name="ot", tag="ot")
                        nc.vector.tensor_tensor(
                            ot[:], po[:], bias_bc[:, oc2 * 512:(oc2 + 1) * 512],
                            ADD)
                        nc.sync.dma_start(
                            out_d[qc * P:(qc + 1) * P,
                                  oc2 * 512:(oc2 + 1) * 512], ot[:])

    nc.compile()
    return nc


_NC_CACHE = None


def _get_nc():
    global _NC_CACHE
    if _NC_CACHE is None:
        _NC_CACHE = _build()
    return _NC_CACHE


def _run(z, w_q, w_k, w_v, w_o, b_o, **spmd_kwargs):
    z = np.ascontiguousarray(np.asarray(z, dtype=np.float32))
    w_q = np.ascontiguousarray(np.asarray(w_q, dtype=np.float32))
    w_k = np.ascontiguousarray(np.asarray(w_k, dtype=np.float32))
    w_v = np.ascontiguousarray(np.asarray(w_v, dtype=np.float32))
    w_o = np.ascontiguousarray(np.asarray(w_o, dtype=np.float32))
    b_o = np.ascontiguousarray(np.asarray(b_o, dtype=np.float32))
    assert z.shape == (B, N, D)

    if not spmd_kwargs.get("trace"):
        # A stray BASS_TRACE in the environment would route through the NTFF
        # hook (absent in this image) and crash; force the no-trace path.
        os.environ["BASS_NEVER_TRACE"] = "1"

    nc = _get_nc()
    in_maps = []
    for c in range(N_CORES):
        b = c // 2
        off = (c % 2) * NQ
        zc = np.ascontiguousarray(np.concatenate([z[b, off:], z[b, :off]], axis=0))
        in_maps.append({"z": zc, "w_q": w_q, "w_k": w_k, "w_v": w_v,
                        "w_o": w_o, "b_o": b_o})

    res = run_bass_kernel_spmd(nc, in_maps, core_ids=list(range(N_CORES)),
                               **spmd_kwargs)
    out = np.empty((B, N, D), dtype=np.float32)
    for c in range(N_CORES):
        b = c // 2
        off = (c % 2) * NQ
        out[b, off:off + NQ, :] = res.results[c]["out"]
    return out, res


def kernel(z, w_q, w_k, w_v, w_o, b_o):
    out, _ = _run(z, w_q, w_k, w_v, w_o, b_o)
    return out


# revision 23
# speedup vs baseline: 1.2164x; 1.2164x over previous
"""Multi-head self-attention (B=4, N=2048, D=1024, H=16) on 8 trn2 NeuronCores.

Sharding: 8 shards = (batch, query-half).  Core c handles batch c//2 and query
rows [(c%2)*1024, (c%2)*1024+1024).  Each core receives its batch's z with the
rows rolled so that its query rows come first; rolling permutes the key/value
sequence order, which attention output is invariant to.  K/V are computed for
the full 2048-row sequence on both cores of a batch pair (duplicated compute,
no collectives needed).

Per-core kernel (Tile), restructured for PE/ACT overlap:
  - Everything SBUF-resident in bf16 (no DRAM spill of K^T/Q^T).
  - Per head-pair pipeline: projections for pair p are interleaved with
    attention for pair p-1 so the PE never idles while ACT drains the exp
    stream.
  - Scores via 64-row lhsT slices of the pair's K^T (no zero padding);
    per-head psum scores tile [128, 1024] -> one ACT exp instr.
  - PV in natural orientation: lhsT = exp-scores [128 keys, 128 q] slices,
    rhs = [V_h | 1] (65 cols) -> psum [128 q, 65] accumulated over key
    chunks; col 64 is the softmax denominator.  This streams 65 columns per
    accumulation step instead of 1024, halving PE attention work.
  - Normalize with per-partition reciprocal scalars on DVE, final projection
    from a PE re-transpose of the normalized attention output.
"""

import os
import sys

_TRN_REPO = "/opt/trn_rl_repo"
if os.path.isdir(_TRN_REPO) and _TRN_REPO not in sys.path:
    sys.path.insert(0, _TRN_REPO)

import numpy as np

import concourse.bass as bass  # noqa: E402
import concourse.mybir as mybir  # noqa: E402
from concourse import bacc  # noqa: E402
from concourse.bass_utils import run_bass_kernel_spmd  # noqa: E402
from concourse.masks import make_identity  # noqa: E402
from concourse.tile import TileContext  # noqa: E402

F32 = mybir.dt.float32
BF16 = mybir.dt.bfloat16
MULT = mybir.AluOpType.mult
ADD = mybir.AluOpType.add
EXP = mybir.ActivationFunctionType.Exp

N_CORES = 8
B, N, D = 4, 2048, 1024
H, HD = 16, 64
NQ = N // 2  # query rows per core
P = 128
DC = D // P  # 8 din/dout chunks of 128
NKC = N // P  # 16 key chunks of 128
NP = H // 2  # 8 head pairs
SCALE = 1.0 / 8.0  # 1/sqrt(HD)


def _build():
    nc = bacc.Bacc("TRN2", target_bir_lowering=False, debug=False,
                   num_devices=N_CORES)
    z_d = nc.declare_dram_parameter("z", [N, D], F32, isOutput=False)
    wq_d = nc.declare_dram_parameter("w_q", [D, D], F32, isOutput=False)
    wk_d = nc.declare_dram_parameter("w_k", [D, D], F32, isOutput=False)
    wv_d = nc.declare_dram_parameter("w_v", [D, D], F32, isOutput=False)
    wo_d = nc.declare_dram_parameter("w_o", [D, D], F32, isOutput=False)
    bo_d = nc.declare_dram_parameter("b_o", [D], F32, isOutput=False)
    out_d = nc.declare_dram_parameter("out", [NQ, D], F32, isOutput=True)

    with TileContext(nc) as tc:
        with tc.tile_pool(name="const", bufs=1) as constp, \
             tc.tile_pool(name="pers", bufs=1) as persp:
            identf = constp.tile([P, P], F32, name="identf")
            make_identity(nc, identf)
            ident16 = constp.tile([P, P], BF16, name="ident16")
            make_identity(nc, ident16)

            bo_sb = constp.tile([1, D], F32, name="bo_sb")
            nc.sync.dma_start(bo_sb[:], bo_d[None, :])
            bias_bc = constp.tile([P, D], F32, name="bias_bc")
            nc.gpsimd.partition_broadcast(bias_bc[:], bo_sb[:])

            # attention output, natural ([q-part, qc, din]) and transposed
            attnN = persp.tile([P, NQ // P, D], BF16, name="attnN")
            attnT = persp.tile([P, DC, NQ], BF16, name="attnT")
            wo16 = persp.tile([P, DC, D], BF16, name="wo16")

            with tc.tile_pool(name="zts", bufs=1) as ztsp, \
                 tc.tile_pool(name="wpair", bufs=2) as wpairp, \
                 tc.tile_pool(name="wstg", bufs=2) as wstgp, \
                 tc.tile_pool(name="wostg", bufs=1) as wostgp, \
                 tc.tile_pool(name="zin", bufs=2) as zinp, \
                 tc.tile_pool(name="kqv", bufs=1) as kqvp, \
                 tc.tile_pool(name="es", bufs=6) as esp, \
                 tc.tile_pool(name="rec", bufs=4) as recp, \
                 tc.tile_pool(name="psproj", bufs=2, space="PSUM") as projps, \
                 tc.tile_pool(name="psscore", bufs=2, space="PSUM") as scoreps, \
                 tc.tile_pool(name="pspv", bufs=2, space="PSUM") as pvps:

                zT = ztsp.tile([P, DC, N], BF16, name="zT")

                # persistent, manually double-buffered K^T/Q^T/V' tiles.
                # K^T/Q^T are per head, zero-padded to 128 contraction rows
                # (head 0 at partitions 0:64, head 1 at 64:128); the pad and
                # the V' ones-column are memset ONCE here, off the critical
                # path, instead of per pair.
                kbuf = [[kqvp.tile([P, N], BF16, name="kb", tag=f"kb{i}{hh}")
                         for hh in range(2)] for i in range(2)]
                qbuf = [[kqvp.tile([P, NQ], BF16, name="qb", tag=f"qb{i}{hh}")
                         for hh in range(2)] for i in range(2)]
                vbuf = [kqvp.tile([P, NKC, 2, HD + 1], BF16, name="vb",
                                  tag=f"vb{i}") for i in range(2)]
                for i in range(2):
                    nc.gpsimd.memset(kbuf[i][0][HD:P, :], 0.0)
                    nc.gpsimd.memset(kbuf[i][1][0:HD, :], 0.0)
                    nc.gpsimd.memset(qbuf[i][0][HD:P, :], 0.0)
                    nc.gpsimd.memset(qbuf[i][1][0:HD, :], 0.0)
                    nc.gpsimd.memset(vbuf[i][:, :, :, HD], 1.0)

                # ---- z load + cast + bf16 transpose into zT ----
                def zt_unit(ch):
                    def f():
                        zin = zinp.tile([P, D], F32, name="zin", tag="zin")
                        nc.sync.dma_start(zin[:], z_d[ch * P:(ch + 1) * P, :])
                        zc = zinp.tile([P, D], BF16, name="zc", tag="zc")
                        nc.scalar.activation(
                            zc[:], zin[:], mybir.ActivationFunctionType.Copy)
                        for dg in range(2):
                            ps = projps.tile([P, 4, P], BF16, name="zps",
                                             tag="pp")
                            for d4 in range(4):
                                dc = dg * 4 + d4
                                nc.tensor.transpose(
                                    ps[:, d4, :],
                                    zc[:, dc * P:(dc + 1) * P],
                                    ident16[:])
                            nc.vector.tensor_copy(
                                zT[:, dg * 4:(dg + 1) * 4,
                                   ch * P:(ch + 1) * P],
                                ps[:])
                    return f

                # ---------- per-pair unit emitters ----------
                def make_proj_units(p, state):
                    """Projection chains for pair p (emitted lazily), fed by
                    per-pair weight column slices so pair 0 starts early."""
                    units = []
                    kTh, qTh = kbuf[p % 2], qbuf[p % 2]
                    vONp = vbuf[p % 2]
                    state[p] = (kTh, qTh, vONp)
                    w16 = {}

                    def w_load(nm, src_d):
                        def f():
                            stg = wstgp.tile([P, DC, P], F32, name="wstg",
                                             tag="wstg")
                            nc.sync.dma_start(
                                stg[:],
                                src_d[:, p * P:(p + 1) * P].rearrange(
                                    "(c p) o -> p c o", p=P))
                            w16[nm] = wpairp.tile([P, DC, P], BF16,
                                                  name="w16", tag=f"w{nm}")
                            nc.vector.tensor_copy(w16[nm][:], stg[:])
                        return f

                    def k_chain(s5):
                        def f():
                            ps = projps.tile([P, 512], F32, name="kps",
                                             tag="pp")
                            for dc in range(DC):
                                nc.tensor.matmul(
                                    ps[:],
                                    lhsT=w16["k"][:, dc, :],
                                    rhs=zT[:, dc, s5 * 512:(s5 + 1) * 512],
                                    start=(dc == 0), stop=(dc == DC - 1))
                            nc.vector.tensor_copy(
                                kTh[0][0:HD, s5 * 512:(s5 + 1) * 512],
                                ps[0:HD, :])
                            nc.vector.tensor_copy(
                                kTh[1][HD:P, s5 * 512:(s5 + 1) * 512],
                                ps[HD:P, :])
                        return f

                    def q_chain(s5):
                        def f():
                            ps = projps.tile([P, 512], F32, name="qps",
                                             tag="pp")
                            for dc in range(DC):
                                nc.tensor.matmul(
                                    ps[:],
                                    lhsT=w16["q"][:, dc, :],
                                    rhs=zT[:, dc, s5 * 512:(s5 + 1) * 512],
                                    start=(dc == 0), stop=(dc == DC - 1))
                            nc.vector.tensor_copy(
                                qTh[0][0:HD, s5 * 512:(s5 + 1) * 512],
                                ps[0:HD, :])
                            nc.vector.tensor_copy(
                                qTh[1][HD:P, s5 * 512:(s5 + 1) * 512],
                                ps[HD:P, :])
                        return f

                    def v_group(g):
                        def f():
                            vt = projps.tile([P, 4, P], F32, name="vps",
                                             tag="pp")
                            for kc4 in range(4):
                                kc = g * 4 + kc4
                                for dc in range(DC):
                                    nc.tensor.matmul(
                                        vt[:, kc4, :],
                                        lhsT=zT[:, dc, kc * P:(kc + 1) * P],
                                        rhs=w16["v"][:, dc, :],
                                        start=(dc == 0), stop=(dc == DC - 1))
                            nc.vector.tensor_copy(
                                vONp[:, g * 4:(g + 1) * 4, :, 0:HD],
                                vt.rearrange("p k (h d) -> p k h d", d=HD))
                        return f

                    units.append(w_load("k", wk_d))
                    units.append(w_load("q", wq_d))
                    units.append(w_load("v", wv_d))
                    for s5 in range(4):
                        units.append(k_chain(s5))
                    for s5 in range(2):
                        units.append(q_chain(s5))
                    for g in range(4):
                        units.append(v_group(g))
                    return units

                def make_attn_units(p, state):
                    """Attention stream for pair p: per head 16 x (S, exp,
                    PV one kc behind) + normalize, then attnT transposes for
                    this pair's din columns."""
                    kTh, qTh, vONp = state[p]
                    units = []

                    def pv_step(kc, hh, pv_tiles, es_tiles):
                        es = es_tiles.pop(kc)
                        for qc in range(8):
                            # one accumulation buffer per psum bank: only
                            # the bank's first chain may zero it (start);
                            # siblings accumulate onto the zeroed buffer.
                            nc.tensor.matmul(
                                pv_tiles[qc // 4][:, qc % 4, :],
                                lhsT=es[:, qc * P:(qc + 1) * P],
                                rhs=vONp[:, kc, hh, :],
                                start=(kc == 0 and qc % 4 == 0),
                                stop=(kc == NKC - 1),
                                skip_group_check=True)

                    for hh in range(2):
                        kTp, qTp = kTh[hh], qTh[hh]
                        pv_tiles = []
                        es_tiles = {}

                        def head_setup(pv_tiles=pv_tiles):
                            for _ in range(2):
                                pv_tiles.append(pvps.tile(
                                    [P, 4, HD + 1], F32, name="pv", tag="pv"))

                        def kc_unit(kc, hh=hh, kTp=kTp, qTp=qTp,
                                    pv_tiles=pv_tiles, es_tiles=es_tiles):
                            def f():
                                ps = scoreps.tile([P, NQ], F32, name="sps",
                                                  tag="sc")
                                for qc2 in range(2):
                                    nc.tensor.matmul(
                                        ps[:, qc2 * 512:(qc2 + 1) * 512],
                                        lhsT=kTp[:, kc * P:(kc + 1) * P],
                                        rhs=qTp[:,
                                                qc2 * 512:(qc2 + 1) * 512])
                                es = esp.tile([P, NQ], BF16, name="es",
                                              tag="es")
                                nc.scalar.activation(es[:], ps[:], EXP,
                                                     scale=SCALE)
                                es_tiles[kc] = es
                                if kc > 0:
                                    # PV runs one kc behind its exp so the
                                    # PE never waits on ACT just-in-time
                                    pv_step(kc - 1, hh, pv_tiles, es_tiles)
                            return f

                        def pv_tail(hh=hh, pv_tiles=pv_tiles,
                                    es_tiles=es_tiles):
                            def f():
                                pv_step(NKC - 1, hh, pv_tiles, es_tiles)
                            return f

                        def norm_unit(half, p=p, hh=hh, pv_tiles=pv_tiles):
                            def f():
                                pv = pv_tiles[half]
                                rec = recp.tile([P, 4, 1], F32, name="rec",
                                                tag="rec")
                                nc.vector.reciprocal(
                                    rec[:], pv[:, :, HD:HD + 1])
                                for qc4 in range(4):
                                    qc = half * 4 + qc4
                                    nc.vector.tensor_scalar(
                                        attnN[:, qc,
                                              (2 * p + hh) * HD:
                                              (2 * p + hh + 1) * HD],
                                        pv[:, qc4, 0:HD],
                                        rec[:, qc4, :], None, MULT)
                            return f

                        units.append(head_setup)
                        for kc in range(NKC):
                            units.append(kc_unit(kc))
                        units.append(pv_tail())
                        units.append(norm_unit(0))
                        units.append(norm_unit(1))

                    def at_unit(qg, p=p):
                        def f():
                            tp = projps.tile([P, 4, P], BF16, name="tp",
                                             tag="pp")
                            for q4 in range(4):
                                qc = qg * 4 + q4
                                nc.tensor.transpose(
                                    tp[:, q4, :],
                                    attnN[:, qc, p * P:(p + 1) * P],
                                    ident16[:])
                            nc.vector.tensor_copy(
                                attnT[:, p, qg * 512:(qg + 1) * 512]
                                .rearrange("p (q c) -> p q c", c=P),
                                tp[:])
                        return f

                    units.append(at_unit(0))
                    units.append(at_unit(1))
                    return units

                # ---------- pipelined emission ----------
                state = {}
                zt_units = [zt_unit(ch) for ch in range(16)]
                for p in range(NP + 1):
                    proj_units = make_proj_units(p, state) if p < NP else []
                    attn_units = make_attn_units(p - 1, state) if p > 0 else []
                    if not attn_units:
                        # pair 0 fill: z chunks interleaved with pair-0
                        # chains in dependency order (chain s5 needs z
                        # chunks 4*s5..4*s5+3)
                        wl, ch_u = proj_units[0:3], proj_units[3:]
                        K, Q, V = ch_u[0:4], ch_u[4:6], ch_u[6:10]
                        order = ([wl[0]] + zt_units[0:4] + [K[0], wl[1]] +
                                 zt_units[4:8] + [K[1], Q[0], wl[2]] +
                                 zt_units[8:12] + [Q[1], V[0], K[2], V[1]] +
                                 zt_units[12:16] +
                                 [K[3], V[2], V[3]])
                        for u in order:
                            u()
                        continue
                    # interleave: sprinkle proj units evenly through the
                    # (longer) attention stream
                    na, npj = len(attn_units), len(proj_units)
                    pi = 0
                    for i, u in enumerate(attn_units):
                        u()
                        want = ((i + 1) * npj) // na
                        while pi < want:
                            proj_units[pi]()
                            pi += 1
                    while pi < npj:
                        proj_units[pi]()
                        pi += 1
                    if p == 5:
                        # preload + cast w_o while attention still runs
                        for half in range(2):
                            stg = wostgp.tile([P, DC // 2, D], F32,
                                              name="wostg", tag="wostg")
                            nc.sync.dma_start(
                                stg[:],
                                wo_d[half * 512:(half + 1) * 512, :]
                                .rearrange("(c p) o -> p c o", p=P))
                            nc.vector.tensor_copy(
                                wo16[:, half * 4:(half + 1) * 4, :], stg[:])

            # ---------------- tail: final projection ----------
            with tc.tile_pool(name="ot", bufs=4) as outp, \
                 tc.tile_pool(name="psf", bufs=2, space="PSUM") as fpp:
                for qc in range(NQ // P):
                    for oc2 in range(2):
                        po = fpp.tile([P, 512], F32, name="po", tag="po")
                        for dc in range(DC):
                            nc.tensor.matmul(
                                po[:],
                                lhsT=attnT[:, dc, qc * P:(qc + 1) * P],
                                rhs=wo16[:, dc, oc2 * 512:(oc2 + 1) * 512],
                                start=(dc == 0), stop=(dc == DC - 1))
                        ot = outp.tile([P, 512], F32, name="ot", tag="ot")
                        nc.vector.tensor_tensor(
                            ot[:], po[:], bias_bc[:, oc2 * 512:(oc2 + 1) * 512],
                            ADD)
                        nc.sync.dma_start(
                            out_d[qc * P:(qc + 1) * P,
                                  oc2 * 512:(oc2 + 1) * 512], ot[:])

    nc.compile()
    return nc


_NC_CACHE = None


def _get_nc():
    global _NC_CACHE
    if _NC_CACHE is None:
        _NC_CACHE = _build()
    return _NC_CACHE


def _run(z, w_q, w_k, w_v, w_o, b_o, **spmd_kwargs):
    z = np.ascontiguousarray(np.asarray(z, dtype=np.float32))
    w_q = np.ascontiguousarray(np.asarray(w_q, dtype=np.float32))
    w_k = np.ascontiguousarray(np.asarray(w_k, dtype=np.float32))
    w_v = np.ascontiguousarray(np.asarray(w_v, dtype=np.float32))
    w_o = np.ascontiguousarray(np.asarray(w_o, dtype=np.float32))
    b_o = np.ascontiguousarray(np.asarray(b_o, dtype=np.float32))
    assert z.shape == (B, N, D)

    if not spmd_kwargs.get("trace"):
        # A stray BASS_TRACE in the environment would route through the NTFF
        # hook (absent in this image) and crash; force the no-trace path.
        os.environ["BASS_NEVER_TRACE"] = "1"

    nc = _get_nc()
    in_maps = []
    for c in range(N_CORES):
        b = c // 2
        off = (c % 2) * NQ
        zc = np.ascontiguousarray(np.concatenate([z[b, off:], z[b, :off]], axis=0))
        in_maps.append({"z": zc, "w_q": w_q, "w_k": w_k, "w_v": w_v,
                        "w_o": w_o, "b_o": b_o})

    res = run_bass_kernel_spmd(nc, in_maps, core_ids=list(range(N_CORES)),
                               **spmd_kwargs)
    out = np.empty((B, N, D), dtype=np.float32)
    for c in range(N_CORES):
        b = c // 2
        off = (c % 2) * NQ
        out[b, off:off + NQ, :] = res.results[c]["out"]
    return out, res


def kernel(z, w_q, w_k, w_v, w_o, b_o):
    out, _ = _run(z, w_q, w_k, w_v, w_o, b_o)
    return out
